# revision 42
# baseline (speedup 1.0000x reference)
"""Trainium2 Bass kernel for a 4-layer GNN-style MLP (ChebConv K=1) with
training-mode BatchNorm, global_add_pool over 64 graphs, and a 3-layer FC head.

Strategy (8 NeuronCores, data-parallel over nodes):
  - 12500 nodes/core, feature-major layout [feat_part(128) x nodes_free] so the
    whole matmul chain needs no transposes.
  - BatchNorm (batch statistics over all 100k nodes) is folded into the next
    matmul's weights: bn(h) @ w + b == h @ (s*w) + (t@w + b).  Per-feature
    sum/sumsq are accumulated on-chip and combined across cores with small
    AllReduces.
  - Engine balance per node-tile keeps the PE free of drain stalls (which
    would hold the PE at its low p-state clock): ACT drains 3 of 4 relu
    chunks (with sum accumulators), DVE drains 1 chunk and computes sumsq
    via tensor_tensor_reduce on the bf16 activations.
  - L0 (128->512) and L1 (512->512) are fused into one software-pipelined
    tile loop (L0 of tile j runs while L1 of tile j-1 drains).
  - Pooling: per-tile node sums come free from the relu accumulators;
    graph-boundary suffixes are corrected with per-tile 0/1 masks
    (tensor_tensor_reduce), then a tiny one-hot matmul scatters tile sums
    into the 64 graph bins; bn3's affine is applied post-AllReduce.
"""

import contextlib

import numpy as np

import concourse.bass as bass
import concourse.tile as tile
from concourse import bacc, mybir
from concourse import bass_utils

F32 = mybir.dt.float32
BF16 = mybir.dt.bfloat16

# Problem constants (hardcoded per contract).
N = 100000          # nodes
IN = 128            # input features
D = 512             # hidden dim
G = 64              # graphs
C = 10              # classes
EPS = 1e-5
NCORES = 8
NS = N // NCORES    # nodes per core = 12500
NT = 500            # node tile (free dim per matmul)
NTILES = NS // NT   # 25
KC = D // 128       # 4 chunks of the hidden dim
FN = float(N)
XSP = 1250          # x streaming span
NXSP = NS // XSP    # 5

AR_GROUPS = [list(range(NCORES))]
KMIX = 1  # mixed (multi-graph) tiles per core; set by build_program

import os
_V = set(os.environ.get("KERNEL_V", "").split(",")) - {""}
# tensor_tensor_reduce hangs TRN2 hardware in this codegen path (bisected
# 2026-08-08); default to the scalar_tensor_tensor fallback.
USE_TTR = "ttr" in _V
FUSE_P1 = "nofuse" not in _V     # pipeline L0+L1 in one tile loop
ALT_DMA_Q = "syncdma" not in _V  # consts on gpsimd DMA queue
POOL_STT = False  # walrus rejects TensorScalarPtr on the Pool engine

Relu = mybir.ActivationFunctionType.Relu
Copy = mybir.ActivationFunctionType.Copy
Sqrt = mybir.ActivationFunctionType.Sqrt
Sig = mybir.ActivationFunctionType.Sigmoid
ADD = mybir.AluOpType.add
MULT = mybir.AluOpType.mult
MAX = mybir.AluOpType.max
AXX = mybir.AxisListType.X


def _bcast_part(ap, nparts):
    """Stride-0 partition broadcast of a DRAM AP: [a, b] -> [nparts, a, b]."""
    return bass.AP(tensor=ap.tensor, offset=ap.offset,
                   ap=[[0, nparts]] + list(ap.ap))


def _build_host_inputs(inputs):
    """Shard + reshape the full problem inputs into per-core input maps.

    Nodes are permuted so that all but K tiles per core hold nodes of a
    single graph (pure tiles -> one-hot row in `arm`); the graph-remainder
    nodes are packed into K "mixed" tiles per core, pooled on-device via a
    node-level one-hot matmul (`oh`).  This removes the suffix-mask pass.
    """
    x = np.asarray(inputs["x"], np.float32)
    batch = np.asarray(inputs["batch"]).astype(np.int64)

    # bn1 is a pure function of the raw input -> fold it into x host-side
    # (the AllReduce + stats pass for bn1 dominated device-side startup).
    xm = x.mean(0, dtype=np.float64)
    xv = x.var(0, dtype=np.float64)
    s1 = (np.asarray(inputs["bn1_g"], np.float64) / np.sqrt(xv + EPS))
    t1 = np.asarray(inputs["bn1_b"], np.float64) - xm * s1
    x = (x * s1 + t1).astype(np.float32)

    counts = np.bincount(batch, minlength=G).astype(np.float32).reshape(1, G)

    # node indices per graph (batch is sorted)
    starts = np.searchsorted(batch, np.arange(G + 1))
    pure_tiles = []   # (graph, node_index_array)
    rem_idx = []
    for g in range(G):
        idx = np.arange(starts[g], starts[g + 1])
        nfull = len(idx) // NT
        for t in range(nfull):
            pure_tiles.append((g, idx[t * NT:(t + 1) * NT]))
        rem_idx.append(idx[nfull * NT:])
    rem = np.concatenate(rem_idx)
    assert len(rem) % NT == 0
    mixed_tiles = [rem[i * NT:(i + 1) * NT] for i in range(len(rem) // NT)]
    M = len(mixed_tiles)
    K = (M + NCORES - 1) // NCORES
    # reclassify pure tiles as mixed to give every core exactly K mixed
    while len(mixed_tiles) < K * NCORES:
        g, idx = pure_tiles.pop()
        mixed_tiles.append(idx)
    npure = NTILES - K

    def chunk_cols(v, nch):
        # [nch*128] -> [128, nch] with chunk c in column c
        return np.ascontiguousarray(np.asarray(v, np.float32).reshape(nch, 128).T)

    common = {
        "w0": np.asarray(inputs["w0"], np.float32),          # [128, 512]
        "w1": np.asarray(inputs["w1"], np.float32),          # [512, 512]
        "w2": np.asarray(inputs["w2"], np.float32),
        "w3": np.asarray(inputs["w3"], np.float32),
        "fc1w": np.asarray(inputs["fc1_w"], np.float32),     # [512, 512]
        "fc2w": np.asarray(inputs["fc2_w"], np.float32),     # [512, 256]
        "fc3w": np.asarray(inputs["fc3_w"], np.float32),     # [256, 10]
        "b0c": chunk_cols(inputs["b0"], KC),
        "bb3r": np.asarray(inputs["bb3"], np.float32).reshape(1, D),
        "bb1c": chunk_cols(inputs["bb1"], KC),
        "bb2c": chunk_cols(inputs["bb2"], KC),
        "bb3c": chunk_cols(inputs["bb3"], KC),
        "fc1bc": chunk_cols(inputs["fc1_b"], KC),
        "fc2bc": chunk_cols(inputs["fc2_b"], 2),
        "fc3bc": np.asarray(inputs["fc3_b"], np.float32).reshape(C, 1),
        "bn3g": chunk_cols(inputs["bn3_g"], KC),
        "bn3b": chunk_cols(inputs["bn3_b"], KC),
        "a3v": np.asarray(inputs["a3"], np.float32).reshape(1, 1),
        "cnts": counts,
    }

    import ml_dtypes
    in_maps = []
    for c in range(NCORES):
        cp = pure_tiles[c * npure:(c + 1) * npure]
        cm = mixed_tiles[c * K:(c + 1) * K]
        arm = np.zeros((npure, G), np.float32)
        node_idx = []
        # mixed tiles first (their pooling work overlaps the pure L3 tiles)
        oh = np.zeros((125, K, KC, G), np.float32)
        for k, idx in enumerate(cm):
            node_idx.append(idx)
            gs = batch[idx]
            for pc in range(KC):
                seg = gs[pc * 125:(pc + 1) * 125]
                oh[np.arange(125), k, pc, seg] = 1.0
        for t, (g, idx) in enumerate(cp):
            arm[t, g] = 1.0
            node_idx.append(idx)
        perm = np.concatenate(node_idx)
        xt = np.ascontiguousarray(x[perm].T).astype(ml_dtypes.bfloat16)
        m = dict(common)
        m["xT"] = xt
        m["arm"] = arm
        m["oh"] = oh.astype(ml_dtypes.bfloat16)
        in_maps.append(m)
    return in_maps, K


def _declare_io(nc):
    specs = {
        "xT": ([IN, NS], BF16),
        "w0": ([IN, D], F32),
        "w1": ([D, D], F32),
        "w2": ([D, D], F32),
        "w3": ([D, D], F32),
        "fc1w": ([D, D], F32),
        "fc2w": ([D, 256], F32),
        "fc3w": ([256, C], F32),
        "b0c": ([128, KC], F32),
        "bb3r": ([1, D], F32),
        "bb1c": ([128, KC], F32),
        "bb2c": ([128, KC], F32),
        "bb3c": ([128, KC], F32),
        "fc1bc": ([128, KC], F32),
        "fc2bc": ([128, 2], F32),
        "fc3bc": ([C, 1], F32),
        "bn3g": ([128, KC], F32),
        "bn3b": ([128, KC], F32),
        "a3v": ([1, 1], F32),
        "cnts": ([1, G], F32),
        "arm": ([NTILES - KMIX, G], F32),
        "oh": ([125, KMIX, KC, G], BF16),
    }
    ins = {k: nc.dram_tensor(k, shape, dt, kind="ExternalInput").ap()
           for k, (shape, dt) in specs.items()}
    out = nc.dram_tensor("out", [C, G], F32, kind="ExternalOutput").ap()
    return ins, out


def build_program(kmix):
    global KMIX
    KMIX = kmix
    nc = bacc.Bacc("TRN2", target_bir_lowering=False, debug=False,
                   enable_asserts=False, num_devices=NCORES)
    ins, out_ap = _declare_io(nc)
    with tile.TileContext(nc) as tc:
        _emit_kernel(nc, tc, ins, out_ap)
    nc.compile()
    return nc


def _emit_kernel(nc, tc, ins, out_ap):
    ctx = contextlib.ExitStack()
    with ctx:
        sbuf = ctx.enter_context(tc.tile_pool(name="sbuf", bufs=1))
        scratch = ctx.enter_context(tc.tile_pool(name="scratch", bufs=3))
        psum = ctx.enter_context(tc.tile_pool(name="psum", bufs=5, space="PSUM"))
        paux = ctx.enter_context(tc.tile_pool(name="paux", bufs=1, space="PSUM"))
        dram = ctx.enter_context(tc.tile_pool(name="dram", bufs=1, space="DRAM"))
        # weight staging ring: w1 -> w2 -> w3 -> fc1w reuse one 8KB slot, so
        # each load's DMA dispatches as soon as the previous tenant was cast
        # (mid-layer), never against a barrier.
        wstage = ctx.enter_context(tc.tile_pool(name="wstage", bufs=1))

        def stats_prepack(sums_t, sq_t, nch, nsum, nsq):
            """Partial stats reduce over all but the last two tile/span
            columns, pre-scaled by 1/N.  Emitted two tiles before the layer
            ends so it clears the DVE queue well before the pack."""
            pp = sbuf.tile([128, nch, 2], F32, tag="statpp", name="pp")
            nc.vector.tensor_reduce(out=pp[:, :, 0], in_=sums_t[:, :, 0:nsum],
                                    axis=AXX, op=ADD)
            nc.vector.tensor_reduce(out=pp[:, :, 1], in_=sq_t[:, :, 0:nsq],
                                    axis=AXX, op=ADD)
            nc.vector.tensor_scalar_mul(out=pp, in0=pp, scalar1=1.0 / FN)
            return pp

        def stats_allreduce(pp, sums_t, sq_t, nch, nsum, nsq):
            """pack = pp + last 2 columns/N; AllReduce(mean, E[x^2])."""
            pack = sbuf.tile([128, nch, 2], F32, tag="statpack", name="pack")
            nc.vector.tensor_add(pack[:, :, 0], sums_t[:, :, nsum],
                                 sums_t[:, :, nsum + 1])
            nc.vector.tensor_add(pack[:, :, 1], sq_t[:, :, nsq],
                                 sq_t[:, :, nsq + 1])
            nc.vector.scalar_tensor_tensor(
                out=pack[:, :, 0], in0=pack[:, :, 0], scalar=1.0 / FN,
                in1=pp[:, :, 0], op0=MULT, op1=ADD)
            nc.vector.scalar_tensor_tensor(
                out=pack[:, :, 1], in0=pack[:, :, 1], scalar=1.0 / FN,
                in1=pp[:, :, 1], op0=MULT, op1=ADD)
            cin = dram.tile([128, nch, 2], F32, tag="ccin", name="cin")
            cout = dram.tile([128, nch, 2], F32, tag="ccout", name="cout")
            red = sbuf.tile([128, nch, 2], F32, tag="statred", name="red")
            nc.gpsimd.dma_start(out=cin, in_=pack)
            nc.gpsimd.collective_compute(
                "AllReduce", ADD, replica_groups=AR_GROUPS,
                ins=[cin.opt()], outs=[cout.opt()])
            nc.gpsimd.dma_start(out=red, in_=cout)
            emit_warm_burst(red[:, :, 0])
            return red

        def emit_s_t(red, nch, g_ap, b_ap):
            """s = g*rsqrt(var+eps), t = b - mean*s; red = [mean, E[x^2]]."""
            m = red[:, :, 0]
            v = sbuf.tile([128, nch], F32, tag="st_v", name="v")
            s = sbuf.tile([128, nch], F32, tag="st_s", name="s")
            t = sbuf.tile([128, nch], F32, tag="st_t", name="t")
            nc.vector.tensor_tensor(out=s, in0=m, in1=m, op=MULT)
            nc.vector.tensor_sub(v, red[:, :, 1], s)
            nc.scalar.activation(out=v, in_=v, func=Sqrt,
                                 bias=eps_t[:, 0:1], scale=1.0)
            nc.vector.reciprocal(out=s, in_=v)
            nc.vector.tensor_mul(s, s, g_ap)
            nc.vector.tensor_mul(v, m, s)
            nc.vector.tensor_sub(t, b_ap, v)
            return s, t

        def emit_warm_burst(dep_ap, n=40):
            """Junk matmuls gated on the AllReduce result: they run during
            the post-barrier s/t/cast chain, tripping the HAM activity window
            so the first real matmuls issue at full clock instead of 1.2GHz."""
            jp = psum.tile([128, KC], F32, tag="ps", name="jp")
            for _ in range(n):
                nc.tensor.matmul(jp, lhsT=ident, rhs=dep_ap,
                                 start=True, stop=True)

        def emit_w_cast(w_sb, s, name):
            """wf[:, kc, :] = w_sb[:, kc, :] * s[:, kc] -> bf16 [128, KC, D]."""
            wf = sbuf.tile([128, KC, D], BF16, tag="wf", name=name)
            for kc in range(KC):
                nc.vector.tensor_scalar_mul(
                    out=wf[:, kc, :], in0=w_sb[:, kc, :],
                    scalar1=s[:, kc:kc + 1])
            return wf

        def emit_bias_fold(wbf_chunks, t_ap, add_bias_ap, tag):
            """b' = t @ w + bias as [128, KC] via tiny bf16 PE matvecs."""
            nk = len(wbf_chunks)
            t_bf = sbuf.tile([128, nk], BF16, tag="tbf", name="t_bf")
            nc.vector.tensor_copy(out=t_bf, in_=t_ap)
            psb = paux.tile([128, KC], F32, tag="psb", name="psb")
            for dc in range(KC):
                for kc in range(nk):
                    nc.tensor.matmul(
                        psb[:, dc:dc + 1],
                        lhsT=wbf_chunks[kc][:, dc * 128:(dc + 1) * 128],
                        rhs=t_bf[:, kc:kc + 1],
                        start=(kc == 0), stop=(kc == nk - 1))
            bf = sbuf.tile([128, KC], F32, tag=tag, name=tag + "_bf")
            nc.vector.tensor_add(bf, psb, add_bias_ap)
            return bf

        def layer_matmuls(wf, j):
            """16 matmuls for node tile j: z[dc] = sum_kc wf[kc,dc].T @ R[kc,j]"""
            jsl = slice(j * NT, (j + 1) * NT)
            ps = []
            for dc in range(KC):
                p = psum.tile([128, NT], F32, tag="ps", name="p")
                for kc in range(KC):
                    nc.tensor.matmul(
                        p, lhsT=wf[:, kc, dc * 128:(dc + 1) * 128],
                        rhs=R[:, kc, jsl], start=(kc == 0), stop=(kc == KC - 1))
                ps.append(p)
            return ps

        def emit_mulreduce(in0, in1, accum, width, eng=None):
            """accum = sum(in0 * in1) along free axis (stt with dump)."""
            eng = eng or nc.vector
            dmp = scratch.tile([128, width], BF16, tag="qdump", name="dmp")
            eng.scalar_tensor_tensor(
                out=dmp[:, 0:width], in0=in0, scalar=1.0, in1=in1,
                op0=MULT, op1=MULT, accum_out=accum)

        def layer_drains(ps, bias_ap, Tacc, j, nact=3, acc_j=None):
            """Drain 4 psum chunks: relu+bias -> R[:, dc, jsl] (in place);
            ACT takes the first `nact` chunks (with sum accum), DVE the rest
            (stt with accum)."""
            jj = j if acc_j is None else acc_j
            jsl = slice(j * NT, (j + 1) * NT)
            for dc in range(nact):
                acc = Tacc[:, dc, jj:jj + 1] if Tacc is not None else None
                nc.scalar.activation(
                    out=R[:, dc, jsl], in_=ps[dc], func=Relu,
                    bias=bias_ap[:, dc:dc + 1], scale=1.0, accum_out=acc)
            for dc in range(nact, KC):
                acc = Tacc[:, dc, jj:jj + 1] if Tacc is not None else None
                nc.vector.scalar_tensor_tensor(
                    out=R[:, dc, jsl], in0=ps[dc],
                    scalar=bias_ap[:, dc:dc + 1], in1=zeros_t,
                    op0=ADD, op1=MAX, accum_out=acc)

        def emit_q_spans(j, Qacc, base=0, ntiles=NTILES):
            """After tile j's drains: sumsq over the completed 2-tile span."""
            jj = j - base
            if jj % 2 == 1 or jj == ntiles - 1:
                sp = jj // 2
                a = (j - 1) * NT if jj % 2 == 1 else j * NT
                b = (j + 1) * NT
                for dc in range(KC):
                    emit_mulreduce(R[:, dc, a:b], R[:, dc, a:b],
                                   Qacc[:, dc, sp:sp + 1], b - a)

        NQSP = (NTILES + 1) // 2  # 13 sumsq spans per layer

        # ---------- resident hidden buffer (bf16, holds r0 -> r1 -> r2 -> r3)
        R = sbuf.tile([128, KC, NS], BF16, tag="R", name="R")

        # ---------- constants + activation-table preload ---------------------
        eps_t = sbuf.tile([128, 1], F32, tag="eps", name="eps_t")
        nc.vector.memset(eps_t, EPS)
        zeros_t = sbuf.tile([128, NT], F32, tag="zeros", name="zeros_t")
        nc.vector.memset(zeros_t, 0.0)
        ident = sbuf.tile([128, 128], F32, tag="ident", name="ident")
        identG = sbuf.tile([G, G], F32, tag="identG", name="identG")
        identGb = sbuf.tile([G, G], BF16, tag="identGb", name="identGb")
        ones_t = sbuf.tile([128, 128], BF16, tag="ones", name="ones_t")
        # only Relu+Sqrt tables up front (Sigmoid is prewarmed mid-L3);
        # extra preloads would stall ACT right when the first drains arrive.
        for fn in (Relu, Sqrt):
            dmy = scratch.tile([128, 1], F32, tag="dummy", name="dmy")
            nc.scalar.activation(out=dmy, in_=eps_t, func=fn)


        qconst = nc.gpsimd if ALT_DMA_Q else nc.sync

        def load_const(key, shape, tag):
            t = sbuf.tile(shape, F32, tag=tag, name=tag)
            qconst.dma_start(out=t, in_=ins[key])
            return t

        # ================= P0: x streaming (bn1 folded on host) ==============
        # Sync queue priority: w0 -> x span0 -> w1 -> x spans 1-4 (the first
        # matmul gates on w0 + span0 only); consts on GpSimd (b0c first, it
        # gates the first drains).
        with tc.tile_pool(name="w01pool", bufs=1) as w01pool, \
             tc.tile_pool(name="psumB", bufs=2, space="PSUM") as psumB:
            x_bf = w01pool.tile([128, NS], BF16, tag="xbf", name="x_bf")
            nc.sync.dma_start(out=x_bf[:, 0:NT], in_=ins["xT"][:, 0:NT])
            w0_sb = w01pool.tile([128, D], F32, tag="w0", name="w0_sb")
            nc.sync.dma_start(out=w0_sb, in_=ins["w0"])
            nc.sync.dma_start(out=x_bf[:, NT:XSP], in_=ins["xT"][:, NT:XSP])
            nc.sync.dma_start(out=x_bf[:, XSP:2 * XSP],
                              in_=ins["xT"][:, XSP:2 * XSP])
            w1_sb = wstage.tile([128, KC, D], F32, tag="wst", name="w1_sb")
            for kc in range(KC):
                nc.sync.dma_start(out=w1_sb[:, kc, :],
                                  in_=ins["w1"][kc * 128:(kc + 1) * 128, :])
            for sp in range(2, NXSP):
                a = sp * XSP
                nc.sync.dma_start(out=x_bf[:, a:a + XSP],
                                  in_=ins["xT"][:, a:a + XSP])

            b0c = load_const("b0c", [128, KC], "b0c")
            bb1c = load_const("bb1c", [128, KC], "bb1c")
            bn3g = load_const("bn3g", [128, KC], "bn3g")
            bn3b = load_const("bn3b", [128, KC], "bn3b")
            bb2c = load_const("bb2c", [128, KC], "bb2c")
            bb3c = load_const("bb3c", [128, KC], "bb3c")

            w0f = w01pool.tile([128, D], BF16, tag="wbf0", name="w0f")
            for dc in range(KC):
                nc.vector.tensor_copy(out=w0f[:, dc * 128:(dc + 1) * 128],
                                      in_=w0_sb[:, dc * 128:(dc + 1) * 128])

            # absorb first-collective entry sync (cross-core launch skew)
            # with a throwaway AllReduce fired at t~0 on the GpSimd queue.
            dmy_in = dram.tile([8], F32, tag="dmyi", name="dmy_in")
            dmy_out = dram.tile([8], F32, tag="dmyo", name="dmy_out")
            dmy_sb = scratch.tile([1, 8], F32, tag="dmysb", name="dmy_sb")
            nc.gpsimd.memset(dmy_sb, 0.0)
            nc.gpsimd.dma_start(out=dmy_in.rearrange("(a b) -> a b", a=1),
                                in_=dmy_sb)
            nc.gpsimd.collective_compute(
                "AllReduce", ADD, replica_groups=AR_GROUPS,
                ins=[dmy_in.opt()], outs=[dmy_out.opt()])

            # plain bf16 cast of w1 (layer-1 input r0 has no preceding BN)
            w1f = w01pool.tile([128, KC, D], BF16, tag="w1f", name="w1f")
            for kc in range(KC):
                nc.vector.tensor_copy(out=w1f[:, kc, :], in_=w1_sb[:, kc, :])

            # identities / ones: emitted after the startup-critical DMAs and
            # casts so they stall neither the GpSimd const queue nor the DVE
            # queue ahead of the first drains (used from the barriers on).
            from concourse.masks import make_identity
            make_identity(nc, ident)
            make_identity(nc, identG)
            nc.vector.tensor_copy(out=identGb, in_=identG)
            nc.vector.memset(ones_t, 1.0)

            # stage w2 (slot WAR clears after the w1f cast, still in P0)
            w2_sb = wstage.tile([128, KC, D], F32, tag="wst", name="w2_sb")
            for kc in range(KC):
                nc.sync.dma_start(out=w2_sb[:, kc, :],
                                  in_=ins["w2"][kc * 128:(kc + 1) * 128, :])
            # small fc weights stream during P1 (fresh space, no WAR)
            fc2w_sb = wstage.tile([128, KC, 256], F32, tag="wst2",
                                  name="fc2w_sb")
            for kc in range(KC):
                nc.sync.dma_start(out=fc2w_sb[:, kc, :],
                                  in_=ins["fc2w"][kc * 128:(kc + 1) * 128, :])
            fc3w_sb = wstage.tile([128, 2, C], F32, tag="wst3", name="fc3w_sb")
            for kc in range(2):
                nc.sync.dma_start(out=fc3w_sb[:, kc, :],
                                  in_=ins["fc3w"][kc * 128:(kc + 1) * 128, :])

            # ---- P1: fused L0+L1, software-pipelined by one tile ------------
            T1 = sbuf.tile([128, KC, NTILES], F32, tag="T1", name="T1")
            Q1 = sbuf.tile([128, KC, NQSP], F32, tag="Q1", name="Q1")

            def emit_l0(j):
                jsl = slice(j * NT, (j + 1) * NT)
                ps0 = []
                for dc in range(KC):
                    p = psumB.tile([128, NT], F32, tag="ps0", name="p0")
                    nc.tensor.matmul(
                        p, lhsT=w0f[:, dc * 128:(dc + 1) * 128],
                        rhs=x_bf[:, jsl], start=True, stop=True)
                    ps0.append(p)
                for dc in range(2):
                    nc.scalar.activation(
                        out=R[:, dc, jsl], in_=ps0[dc], func=Relu,
                        bias=b0c[:, dc:dc + 1], scale=1.0)
                for dc in range(2, KC):
                    nc.vector.tensor_scalar(
                        out=R[:, dc, jsl], in0=ps0[dc],
                        scalar1=b0c[:, dc:dc + 1], scalar2=0.0,
                        op0=ADD, op1=MAX)

            def emit_l1(jj):
                ps = layer_matmuls(w1f, jj)
                layer_drains(ps, bb1c, T1, jj,
                             nact=4 if jj == NTILES - 1 else 3)
                emit_q_spans(jj, Q1)

            emit_warm_burst(w0_sb[:, 0:KC], n=40)
            pp1 = None
            if FUSE_P1:
                for j in range(NTILES + 1):
                    if j < NTILES:
                        emit_l0(j)
                    if j >= 1:
                        emit_l1(j - 1)
                    if j == NTILES - 2:
                        pp1 = stats_prepack(T1, Q1, KC, NTILES - 2, NQSP - 2)
            else:
                for j in range(NTILES):
                    emit_l0(j)
                for j in range(NTILES):
                    emit_l1(j)
                pp1 = stats_prepack(T1, Q1, KC, NTILES - 1, NQSP - 1)

        # ================= barrier #2 + L2 ===================================
        red2 = stats_allreduce(pp1, T1, Q1, KC, NTILES - 2, NQSP - 2)
        s2, t2 = emit_s_t(red2, KC, bn3g, bn3b)
        w2f = emit_w_cast(w2_sb, s2, "w2f")
        wbf2 = sbuf.tile([128, KC, D], BF16, tag="wbf", name="wbf2")
        for kc in range(KC):
            nc.vector.tensor_copy(out=wbf2[:, kc, :], in_=w2_sb[:, kc, :])
        b2f = emit_bias_fold([wbf2[:, kc, :] for kc in range(KC)],
                             t2, bb2c, "b2f")
        # stage w3 (slot WAR clears right here, at the w2 casts)
        w3_sb = wstage.tile([128, KC, D], F32, tag="wst", name="w3_sb")
        for kc in range(KC):
            nc.sync.dma_start(out=w3_sb[:, kc, :],
                              in_=ins["w3"][kc * 128:(kc + 1) * 128, :])
        T2 = sbuf.tile([128, KC, NTILES], F32, tag="T2", name="T2")
        Q2 = sbuf.tile([128, KC, NQSP], F32, tag="Q2", name="Q2")
        pp2 = None
        for j in range(NTILES):
            ps = layer_matmuls(w2f, j)
            layer_drains(ps, b2f, T2, j, nact=4 if j == NTILES - 1 else 3)
            emit_q_spans(j, Q2)
            if j == NTILES - 3:
                pp2 = stats_prepack(T2, Q2, KC, NTILES - 2, NQSP - 2)

        # ================= barrier #3 + L3 + pooling + FC head ===============
        NPURE = NTILES - KMIX
        NQ3 = (NPURE + 1) // 2
        with tc.tile_pool(name="fcpool", bufs=1) as fcpool, \
             tc.tile_pool(name="pmix", bufs=1, space="PSUM") as pmix:
            # tail consts on the Sync DMA queue -- the GpSimd queue must stay
            # clear so the stats-AllReduce triggers fire promptly.
            arm_sb = fcpool.tile([NPURE, G], F32, tag="arm", name="arm_sb")
            nc.sync.dma_start(out=arm_sb, in_=ins["arm"])
            oh_sb = fcpool.tile([125, KMIX, KC, G], BF16, tag="oh", name="oh_sb")
            nc.sync.dma_start(out=oh_sb, in_=ins["oh"])
            ngb = fcpool.tile([128, G], F32, tag="ngb", name="ngb")
            nc.sync.dma_start(out=ngb, in_=ins["cnts"].to_broadcast([128, G]))
            a3b = fcpool.tile([128, 1], F32, tag="a3b", name="a3b")
            nc.sync.dma_start(out=a3b, in_=ins["a3v"].to_broadcast([128, 1]))
            fc1bc_sb = fcpool.tile([128, KC], F32, tag="fc1bc", name="fc1bc_sb")
            nc.sync.dma_start(out=fc1bc_sb, in_=ins["fc1bc"])
            fc2bc_sb = fcpool.tile([128, 2], F32, tag="fc2bc", name="fc2bc_sb")
            nc.sync.dma_start(out=fc2bc_sb, in_=ins["fc2bc"])
            fc3bc_sb = fcpool.tile([C, 1], F32, tag="fc3bc", name="fc3bc_sb")
            nc.sync.dma_start(out=fc3bc_sb, in_=ins["fc3bc"])
            bb3r_sb = fcpool.tile([1, D], F32, tag="bb3r", name="bb3r_sb")
            nc.sync.dma_start(out=bb3r_sb, in_=ins["bb3r"])

            red3 = stats_allreduce(pp2, T2, Q2, KC, NTILES - 2, NQSP - 2)
            s3, t3 = emit_s_t(red3, KC, bn3g, bn3b)
            w3f = emit_w_cast(w3_sb, s3, "w3f")
            wbf3 = sbuf.tile([128, KC, D], BF16, tag="wbf", name="wbf3")
            for kc in range(KC):
                nc.vector.tensor_copy(out=wbf3[:, kc, :], in_=w3_sb[:, kc, :])
            b3f = emit_bias_fold([wbf3[:, kc, :] for kc in range(KC)],
                                 t3, bb3c, "b3f")
            # row layout of the same folded bias, for the node-major mixed path
            t3b = fcpool.tile([128, KC], BF16, tag="t3b", name="t3b")
            nc.vector.tensor_copy(out=t3b, in_=t3)
            pbr = pmix.tile([1, D], F32, tag="pbr", name="pbr")
            for kc in range(KC):
                nc.tensor.matmul(pbr, lhsT=t3b[:, kc:kc + 1],
                                 rhs=wbf3[:, kc, :],
                                 start=(kc == 0), stop=(kc == KC - 1))
            b3row = fcpool.tile([1, D], BF16, tag="b3row", name="b3row")
            nc.vector.tensor_tensor(out=b3row, in0=pbr, in1=bb3r_sb, op=ADD)
            # stage fc1w (slot WAR clears at the w3 casts); bf16 cast follows
            fc1w_sb = wstage.tile([128, KC, D], F32, tag="wst", name="fc1w_sb")
            for kc in range(KC):
                nc.sync.dma_start(out=fc1w_sb[:, kc, :],
                                  in_=ins["fc1w"][kc * 128:(kc + 1) * 128, :])

            # ---- L3 mixed tiles: node-major z3^T path (emitted after two
            # dense pure tiles so the PE is back at full clock first) --------
            # z3t[n,f] = sum_kc r2[kc,n]^T w3f[kc,f] (+ 1 x b3row), relu on ACT,
            # then pooled directly per graph (oh) and sumsq via a ones matvec;
            # no transposes, and the whole tail dependence shrinks to the AR.
            mix = {}

            def emit_mixed_phase():
                poolM = pmix.tile([G, D], F32, tag="poolM", name="poolM")
                sqrow = pmix.tile([1, D], F32, tag="pbr", name="sqrow")
                nmm = KMIX * KC
                pend = []

                def emit_mix_pool(p):
                    r3t_, sq_, k_, pc_, i_ = p
                    nc.tensor.matmul(poolM, lhsT=oh_sb[:, k_, pc_, :],
                                     rhs=r3t_,
                                     start=(i_ == 0), stop=(i_ == nmm - 1),
                                     skip_group_check=True)
                    nc.tensor.matmul(sqrow, lhsT=ones_t[0:125, 0:1], rhs=sq_,
                                     start=(i_ == 0), stop=(i_ == nmm - 1),
                                     skip_group_check=True)

                for k in range(KMIX):
                    for pc in range(KC):
                        a = k * NT + pc * 125
                        zt = psum.tile([125, D], F32, tag="ps", name="zt")
                        for kc in range(KC):
                            nc.tensor.matmul(zt, lhsT=R[:, kc, a:a + 125],
                                             rhs=w3f[:, kc, :],
                                             start=(kc == 0), stop=False)
                        nc.tensor.matmul(zt, lhsT=ones_t[0:1, 0:125],
                                         rhs=b3row, start=False, stop=True)
                        r3t = scratch.tile([125, D], BF16, tag="r3t",
                                           name="r3t")
                        nc.scalar.activation(out=r3t, in_=zt, func=Relu,
                                             bias=zeros_t[0:125, 0:1],
                                             scale=1.0)
                        sq = scratch.tile([125, D], BF16, tag="sqm", name="sq")
                        nc.vector.tensor_tensor(out=sq, in0=r3t, in1=r3t,
                                                op=MULT)
                        pend.append((r3t, sq, k, pc, k * KC + pc))
                        if len(pend) > 2:
                            emit_mix_pool(pend.pop(0))
                for p in pend:
                    emit_mix_pool(p)

                # mixed-node sumsq -> feature-major (runs during pure L3)
                sq_sb = fcpool.tile([1, D], F32, tag="sqsb", name="sq_sb")
                nc.vector.tensor_copy(out=sq_sb, in_=sqrow)
                poolM_bf = fcpool.tile([G, D], F32, tag="poolMsb",
                                       name="poolM_bf")
                nc.vector.tensor_copy(out=poolM_bf, in_=poolM)
                sqT = paux.tile([128, KC], F32, tag="psb", name="sqT")
                for kc in range(KC):
                    nc.tensor.matmul(sqT[:, kc:kc + 1],
                                     lhsT=sq_sb[0:1, kc * 128:(kc + 1) * 128],
                                     rhs=ident[0:1, 0:1], start=True,
                                     stop=True)
                # bf16 fc-weight copies, emitted here so they clear the DVE
                # queue long before the stats pack of the last pure tile
                fc1wb = fcpool.tile([128, KC, D], BF16, tag="fc1wb",
                                    name="fc1wb")
                for kc in range(KC):
                    nc.vector.tensor_copy(out=fc1wb[:, kc, :],
                                          in_=fc1w_sb[:, kc, :])
                fc2wb = fcpool.tile([128, KC, 256], BF16, tag="fc2wb",
                                    name="fc2wb")
                for kc in range(KC):
                    nc.vector.tensor_copy(out=fc2wb[:, kc, :],
                                          in_=fc2w_sb[:, kc, :])
                fc3wb = fcpool.tile([128, 2, C], BF16, tag="fc3wb",
                                    name="fc3wb")
                for kc in range(2):
                    nc.vector.tensor_copy(out=fc3wb[:, kc, :],
                                          in_=fc3w_sb[:, kc, :])
                mix.update(sqT=sqT, poolM_bf=poolM_bf, fc1wb=fc1wb,
                           fc2wb=fc2wb, fc3wb=fc3wb)

            # ---- L3 pure tiles: r3 -> R in place; per-tile sums + sumsq -----
            T3 = sbuf.tile([128, KC, NPURE], F32, tag="TC", name="T3")
            Q3 = sbuf.tile([128, KC, NQ3], F32, tag="Q3", name="Q3")
            pp3 = None
            for jj in range(NPURE):
                if jj == 2:
                    emit_mixed_phase()
                j = KMIX + jj
                ps = layer_matmuls(w3f, j)
                layer_drains(ps, b3f, T3, j, acc_j=jj,
                             nact=4 if jj == NPURE - 1 else 3)
                emit_q_spans(j, Q3, base=KMIX, ntiles=NPURE)
                if jj == 2 * NQ3 - 5:
                    # partial sumsq (pure spans + mixed), pre-scaled by 1/N
                    q1 = sbuf.tile([128, KC], F32, tag="pp3a", name="q1")
                    nc.vector.tensor_reduce(out=q1, in_=Q3[:, :, 0:NQ3 - 2],
                                            axis=AXX, op=ADD)
                    nc.vector.tensor_add(q1, q1, mix['sqT'])
                    pp3 = sbuf.tile([128, KC], F32, tag="pp3", name="pp3")
                    nc.vector.tensor_scalar_mul(out=pp3, in0=q1,
                                                scalar1=1.0 / FN)

            # pooled partials: arm.T @ T3.T (pure, fp32) + poolM folded into
            # the same psum via an identity matmul; psum -> SBUF copies ride
            # the ACT engine (values are sums of relus, so Relu == Copy) and
            # each dc's slice DMAs to the collective buffer as soon as ready.
            nst = 128 * KC
            flat = dram.tile([nst + G * D], F32, tag="cc4in", name="flat")
            flat_out = dram.tile([nst + G * D], F32, tag="cc4out",
                                 name="flat_out")
            flat2 = flat[nst:].rearrange("(g f) -> g f", g=G)
            pack = sbuf.tile([128, KC], F32, tag="statpack", name="pack4")
            nc.vector.tensor_add(pack, Q3[:, :, NQ3 - 2], Q3[:, :, NQ3 - 1])
            nc.vector.scalar_tensor_tensor(
                out=pack, in0=pack, scalar=1.0 / FN,
                in1=pp3, op0=MULT, op1=ADD)
            nc.gpsimd.dma_start(
                out=flat[0:nst].rearrange("(p c) -> p c", p=128), in_=pack)
            poolGc = fcpool.tile([G, D], F32, tag="poolG", name="poolGc")
            for dc in range(KC):
                dsl = slice(dc * 128, (dc + 1) * 128)
                pT = psum.tile([NPURE, 128], F32, tag="ps", name="pT")
                nc.tensor.transpose(pT, T3[:, dc, :], ident)
                tct = scratch.tile([NPURE, 128], F32, tag="tct", name="tct")
                nc.scalar.activation(out=tct, in_=pT, func=Relu)
                pG = psum.tile([G, 128], F32, tag="ps", name="pG")
                nc.tensor.matmul(pG, lhsT=arm_sb, rhs=tct,
                                 start=True, stop=False)
                nc.tensor.matmul(pG, lhsT=identG, rhs=mix['poolM_bf'][:, dsl],
                                 start=False, stop=True)
                nc.scalar.activation(out=poolGc[:, dsl], in_=pG, func=Relu)
                qd = nc.scalar if dc < 2 else nc.sync
                qd.dma_start(out=flat2[:, dsl], in_=poolGc[:, dsl])
            nc.gpsimd.collective_compute(
                "AllReduce", ADD, replica_groups=AR_GROUPS,
                ins=[flat.opt()], outs=[flat_out.opt()])
            red4 = sbuf.tile([128, KC, 2], F32, tag="statred", name="red4")
            poolGr = fcpool.tile([G, D], F32, tag="poolGr", name="poolGr")
            nc.gpsimd.dma_start(
                out=red4[:, :, 1],
                in_=flat_out[0:nst].rearrange("(p c) -> p c", p=128))
            emit_warm_burst(red4[:, :, 1], n=56)
            flat2o = flat_out[nst:].rearrange("(g f) -> g f", g=G)

            # mean comes free from the pooled sums: m = sum_g pooled / N
            pFs = []
            for dc in range(KC):
                dsl = slice(dc * 128, (dc + 1) * 128)
                qd = nc.scalar if dc < 2 else nc.sync
                qd.dma_start(out=poolGr[:, dsl], in_=flat2o[:, dsl])
                pF = psum.tile([128, G], F32, tag="ps", name="pF")
                nc.tensor.transpose(pF, poolGr[:, dsl], identG)
                nc.vector.tensor_reduce(out=red4[:, dc:dc + 1, 0], in_=pF,
                                        axis=AXX, op=ADD)
                pFs.append(pF)
            nc.vector.tensor_scalar_mul(out=red4[:, :, 0], in0=red4[:, :, 0],
                                        scalar1=1.0 / FN)
            s4, t4 = emit_s_t(red4, KC, bn3g, bn3b)
            dmy2 = scratch.tile([128, 1], F32, tag="dummy", name="dmy2")
            nc.scalar.activation(out=dmy2, in_=eps_t, func=Sig)

            # pooled_bn[f, g] = s4[f]*pooled[f, g] + t4[f]*n[g] (feature-major,
            # bf16 for the fc matmuls)
            pooled = fcpool.tile([128, KC, G], BF16, tag="pooled", name="pooled")
            for dc in range(KC):
                ngt = scratch.tile([128, G], F32, tag="ngt", name="ngt")
                nc.vector.tensor_scalar(
                    out=ngt, in0=ngb, scalar1=t4[:, dc:dc + 1],
                    scalar2=None, op0=MULT)
                nc.vector.scalar_tensor_tensor(
                    out=pooled[:, dc, :], in0=pFs[dc],
                    scalar=s4[:, dc:dc + 1], in1=ngt, op0=MULT, op1=ADD)

            # ---------------- FC head (bf16 matmuls) -------------------------
            h1 = fcpool.tile([128, KC, G], BF16, tag="h1", name="h1")
            for dc in range(KC):
                p = psum.tile([128, G], F32, tag="ps", name="pfc")
                for kc in range(KC):
                    nc.tensor.matmul(
                        p, lhsT=mix['fc1wb'][:, kc, dc * 128:(dc + 1) * 128],
                        rhs=pooled[:, kc, :], start=(kc == 0), stop=(kc == KC - 1))
                z = scratch.tile([128, G], F32, tag="fcz", name="z")
                nc.vector.tensor_scalar(out=z, in0=p,
                                        scalar1=fc1bc_sb[:, dc:dc + 1],
                                        scalar2=None, op0=ADD)
                # prelu(z) = max(a*z, z) for 0 <= a <= 1
                nc.vector.scalar_tensor_tensor(
                    out=h1[:, dc, :], in0=z, scalar=a3b[:, 0:1], in1=z,
                    op0=MULT, op1=MAX)

            # fc2 + sigmoid -> h2 [128, 2, G]
            h2 = fcpool.tile([128, 2, G], BF16, tag="h2", name="h2")
            for ec in range(2):
                p = psum.tile([128, G], F32, tag="ps", name="pfc2")
                for kc in range(KC):
                    nc.tensor.matmul(
                        p, lhsT=mix['fc2wb'][:, kc, ec * 128:(ec + 1) * 128],
                        rhs=h1[:, kc, :], start=(kc == 0), stop=(kc == KC - 1))
                nc.scalar.activation(out=h2[:, ec, :], in_=p, func=Sig,
                                     bias=fc2bc_sb[:, ec:ec + 1], scale=1.0)

            # fc3 -> out [10, 64]
            p = psum.tile([C, G], F32, tag="ps", name="pfc3")
            for kc in range(2):
                nc.tensor.matmul(p, lhsT=mix['fc3wb'][:, kc, :], rhs=h2[:, kc, :],
                                 start=(kc == 0), stop=(kc == 1))
            ob = fcpool.tile([C, G], F32, tag="ob", name="ob")
            nc.vector.tensor_scalar(out=ob, in0=p, scalar1=fc3bc_sb,
                                    scalar2=None, op0=ADD)
            nc.sync.dma_start(out=out_ap, in_=ob)


_cached = {}


def kernel(**inputs) -> np.ndarray:
    in_maps, kmix = _build_host_inputs(inputs)
    if _cached.get("kmix") != kmix:
        _cached["nc"] = build_program(kmix)
        _cached["kmix"] = kmix
    nc = _cached["nc"]
    res = bass_utils.run_bass_kernel_spmd(
        nc, in_maps, core_ids=list(range(NCORES)))
    out = res.results[0]["out"]  # [10, 64]
    return np.ascontiguousarray(out.T.astype(np.float32))


if __name__ == "__main__":
    import reference
    inp = {k: np.asarray(v) for k, v in reference.setup_inputs().items()}
    got = kernel(**inp)
    exp = np.asarray(reference.reference(**{
        k: np.asarray(v) for k, v in reference.setup_inputs().items()}))
    err = np.linalg.norm(got - exp) / np.linalg.norm(exp)
    print("Relative error:", err)



# revision 43
# speedup vs baseline: 1.0963x; 1.0963x over previous
"""Trainium2 Bass kernel for a 4-layer GNN-style MLP (ChebConv K=1) with
training-mode BatchNorm, global_add_pool over 64 graphs, and a 3-layer FC head.

Strategy (8 NeuronCores, data-parallel over nodes):
  - 12500 nodes/core, feature-major layout [feat_part(128) x nodes_free] so the
    whole matmul chain needs no transposes.
  - BatchNorm (batch statistics over all 100k nodes) is folded into the next
    matmul's weights: bn(h) @ w + b == h @ (s*w) + (t@w + b).  Per-feature
    sum/sumsq are accumulated on-chip and combined across cores with small
    AllReduces.
  - Engine balance per node-tile keeps the PE free of drain stalls (which
    would hold the PE at its low p-state clock): ACT drains 3 of 4 relu
    chunks (with sum accumulators), DVE drains 1 chunk and computes sumsq
    via tensor_tensor_reduce on the bf16 activations.
  - L0 (128->512) and L1 (512->512) are fused into one software-pipelined
    tile loop (L0 of tile j runs while L1 of tile j-1 drains).
  - Pooling: per-tile node sums come free from the relu accumulators;
    graph-boundary suffixes are corrected with per-tile 0/1 masks
    (tensor_tensor_reduce), then a tiny one-hot matmul scatters tile sums
    into the 64 graph bins; bn3's affine is applied post-AllReduce.
"""

import contextlib

import numpy as np

import concourse.bass as bass
import concourse.tile as tile
from concourse import bacc, mybir
from concourse import bass_utils

F32 = mybir.dt.float32
BF16 = mybir.dt.bfloat16

# Problem constants (hardcoded per contract).
N = 100000          # nodes
IN = 128            # input features
D = 512             # hidden dim
G = 64              # graphs
C = 10              # classes
EPS = 1e-5
NCORES = 8
NS = N // NCORES    # nodes per core = 12500
NT = 500            # node tile (free dim per matmul)
NTILES = NS // NT   # 25
KC = D // 128       # 4 chunks of the hidden dim
FN = float(N)
XSP = 1250          # x streaming span
NXSP = NS // XSP    # 5

AR_GROUPS = [list(range(NCORES))]
KMIX = 1  # mixed (multi-graph) tiles per core; set by build_program

import os
_V = set(os.environ.get("KERNEL_V", "").split(",")) - {""}
# tensor_tensor_reduce hangs TRN2 hardware in this codegen path (bisected
# 2026-08-08); default to the scalar_tensor_tensor fallback.
USE_TTR = "ttr" in _V
FUSE_P1 = "nofuse" not in _V     # pipeline L0+L1 in one tile loop
ALT_DMA_Q = "syncdma" not in _V  # consts on gpsimd DMA queue
POOL_STT = False  # walrus rejects TensorScalarPtr on the Pool engine

Relu = mybir.ActivationFunctionType.Relu
Copy = mybir.ActivationFunctionType.Copy
Sqrt = mybir.ActivationFunctionType.Sqrt
Sig = mybir.ActivationFunctionType.Sigmoid
ADD = mybir.AluOpType.add
MULT = mybir.AluOpType.mult
MAX = mybir.AluOpType.max
AXX = mybir.AxisListType.X


def _bcast_part(ap, nparts):
    """Stride-0 partition broadcast of a DRAM AP: [a, b] -> [nparts, a, b]."""
    return bass.AP(tensor=ap.tensor, offset=ap.offset,
                   ap=[[0, nparts]] + list(ap.ap))


def _build_host_inputs(inputs):
    """Shard + reshape the full problem inputs into per-core input maps.

    Nodes are permuted so that all but K tiles per core hold nodes of a
    single graph (pure tiles -> one-hot row in `arm`); the graph-remainder
    nodes are packed into K "mixed" tiles per core, pooled on-device via a
    node-level one-hot matmul (`oh`).  This removes the suffix-mask pass.
    """
    x = np.asarray(inputs["x"], np.float32)
    batch = np.asarray(inputs["batch"]).astype(np.int64)

    # bn1 is a pure function of the raw input -> fold it into x host-side
    # (the AllReduce + stats pass for bn1 dominated device-side startup).
    xm = x.mean(0, dtype=np.float64)
    xv = x.var(0, dtype=np.float64)
    s1 = (np.asarray(inputs["bn1_g"], np.float64) / np.sqrt(xv + EPS))
    t1 = np.asarray(inputs["bn1_b"], np.float64) - xm * s1
    x = (x * s1 + t1).astype(np.float32)

    counts = np.bincount(batch, minlength=G).astype(np.float32).reshape(1, G)

    # node indices per graph (batch is sorted)
    starts = np.searchsorted(batch, np.arange(G + 1))
    pure_tiles = []   # (graph, node_index_array)
    rem_idx = []
    for g in range(G):
        idx = np.arange(starts[g], starts[g + 1])
        nfull = len(idx) // NT
        for t in range(nfull):
            pure_tiles.append((g, idx[t * NT:(t + 1) * NT]))
        rem_idx.append(idx[nfull * NT:])
    rem = np.concatenate(rem_idx)
    assert len(rem) % NT == 0
    mixed_tiles = [rem[i * NT:(i + 1) * NT] for i in range(len(rem) // NT)]
    M = len(mixed_tiles)
    K = (M + NCORES - 1) // NCORES
    # reclassify pure tiles as mixed to give every core exactly K mixed
    while len(mixed_tiles) < K * NCORES:
        g, idx = pure_tiles.pop()
        mixed_tiles.append(idx)
    npure = NTILES - K

    def chunk_cols(v, nch):
        # [nch*128] -> [128, nch] with chunk c in column c
        return np.ascontiguousarray(np.asarray(v, np.float32).reshape(nch, 128).T)

    common = {
        "w0": np.asarray(inputs["w0"], np.float32),          # [128, 512]
        "w1": np.asarray(inputs["w1"], np.float32),          # [512, 512]
        "w2": np.asarray(inputs["w2"], np.float32),
        "w3": np.asarray(inputs["w3"], np.float32),
        "fc1w": np.asarray(inputs["fc1_w"], np.float32),     # [512, 512]
        "fc2w": np.asarray(inputs["fc2_w"], np.float32),     # [512, 256]
        "fc3w": np.asarray(inputs["fc3_w"], np.float32),     # [256, 10]
        "b0c": chunk_cols(inputs["b0"], KC),
        "bb3r": np.asarray(inputs["bb3"], np.float32).reshape(1, D),
        "bb1c": chunk_cols(inputs["bb1"], KC),
        "bb2c": chunk_cols(inputs["bb2"], KC),
        "bb3c": chunk_cols(inputs["bb3"], KC),
        "fc1bc": chunk_cols(inputs["fc1_b"], KC),
        "fc2bc": chunk_cols(inputs["fc2_b"], 2),
        "fc3bc": np.asarray(inputs["fc3_b"], np.float32).reshape(C, 1),
        "bn3g": chunk_cols(inputs["bn3_g"], KC),
        "bn3b": chunk_cols(inputs["bn3_b"], KC),
        "a3v": np.asarray(inputs["a3"], np.float32).reshape(1, 1),
        "cnts": counts,
    }

    import ml_dtypes
    in_maps = []
    for c in range(NCORES):
        cp = pure_tiles[c * npure:(c + 1) * npure]
        cm = mixed_tiles[c * K:(c + 1) * K]
        arm = np.zeros((npure, G), np.float32)
        node_idx = []
        # mixed tiles first (their pooling work overlaps the pure L3 tiles)
        oh = np.zeros((125, K, KC, G), np.float32)
        for k, idx in enumerate(cm):
            node_idx.append(idx)
            gs = batch[idx]
            for pc in range(KC):
                seg = gs[pc * 125:(pc + 1) * 125]
                oh[np.arange(125), k, pc, seg] = 1.0
        for t, (g, idx) in enumerate(cp):
            arm[t, g] = 1.0
            node_idx.append(idx)
        perm = np.concatenate(node_idx)
        xt = np.ascontiguousarray(x[perm].T).astype(ml_dtypes.bfloat16)
        m = dict(common)
        m["xT"] = xt
        m["arm"] = arm
        m["oh"] = oh.astype(ml_dtypes.bfloat16)
        in_maps.append(m)
    return in_maps, K


def _declare_io(nc):
    specs = {
        "xT": ([IN, NS], BF16),
        "w0": ([IN, D], F32),
        "w1": ([D, D], F32),
        "w2": ([D, D], F32),
        "w3": ([D, D], F32),
        "fc1w": ([D, D], F32),
        "fc2w": ([D, 256], F32),
        "fc3w": ([256, C], F32),
        "b0c": ([128, KC], F32),
        "bb3r": ([1, D], F32),
        "bb1c": ([128, KC], F32),
        "bb2c": ([128, KC], F32),
        "bb3c": ([128, KC], F32),
        "fc1bc": ([128, KC], F32),
        "fc2bc": ([128, 2], F32),
        "fc3bc": ([C, 1], F32),
        "bn3g": ([128, KC], F32),
        "bn3b": ([128, KC], F32),
        "a3v": ([1, 1], F32),
        "cnts": ([1, G], F32),
        "arm": ([NTILES - KMIX, G], F32),
        "oh": ([125, KMIX, KC, G], BF16),
    }
    ins = {k: nc.dram_tensor(k, shape, dt, kind="ExternalInput").ap()
           for k, (shape, dt) in specs.items()}
    out = nc.dram_tensor("out", [C, G], F32, kind="ExternalOutput").ap()
    return ins, out


def build_program(kmix):
    global KMIX
    KMIX = kmix
    nc = bacc.Bacc("TRN2", target_bir_lowering=False, debug=False,
                   enable_asserts=False, num_devices=NCORES)
    ins, out_ap = _declare_io(nc)
    with tile.TileContext(nc) as tc:
        _emit_kernel(nc, tc, ins, out_ap)
    nc.compile()
    return nc


def _emit_kernel(nc, tc, ins, out_ap):
    ctx = contextlib.ExitStack()
    with ctx:
        sbuf = ctx.enter_context(tc.tile_pool(name="sbuf", bufs=1))
        scratch = ctx.enter_context(tc.tile_pool(name="scratch", bufs=3))
        psum = ctx.enter_context(tc.tile_pool(name="psum", bufs=5, space="PSUM"))
        paux = ctx.enter_context(tc.tile_pool(name="paux", bufs=1, space="PSUM"))
        dram = ctx.enter_context(tc.tile_pool(name="dram", bufs=1, space="DRAM"))
        # weight staging ring: w1 -> w2 -> w3 -> fc1w reuse one 8KB slot, so
        # each load's DMA dispatches as soon as the previous tenant was cast
        # (mid-layer), never against a barrier.
        wstage = ctx.enter_context(tc.tile_pool(name="wstage", bufs=1))

        wburst = []

        def stats_prepack(sums_t, sq_t, nch, nsum, nsq):
            """Partial stats reduce over all but the last two tile/span
            columns, pre-scaled by 1/N.  Emitted two tiles before the layer
            ends so it clears the DVE queue well before the pack."""
            pp = sbuf.tile([128, nch, 2], F32, tag="statpp", name="pp")
            nc.vector.tensor_reduce(out=pp[:, :, 0], in_=sums_t[:, :, 0:nsum],
                                    axis=AXX, op=ADD)
            nc.vector.tensor_reduce(out=pp[:, :, 1], in_=sq_t[:, :, 0:nsq],
                                    axis=AXX, op=ADD)
            nc.vector.tensor_scalar_mul(out=pp, in0=pp, scalar1=1.0 / FN)
            return pp

        def stats_allreduce(pp, sums_t, sq_t, nch, nsum, nsq):
            """pack = pp + last 2 columns/N; AllReduce(mean, E[x^2])."""
            pack = sbuf.tile([128, nch, 2], F32, tag="statpack", name="pack")
            nc.vector.tensor_add(pack[:, :, 0], sums_t[:, :, nsum],
                                 sums_t[:, :, nsum + 1])
            nc.vector.tensor_add(pack[:, :, 1], sq_t[:, :, nsq],
                                 sq_t[:, :, nsq + 1])
            nc.vector.scalar_tensor_tensor(
                out=pack[:, :, 0], in0=pack[:, :, 0], scalar=1.0 / FN,
                in1=pp[:, :, 0], op0=MULT, op1=ADD)
            nc.vector.scalar_tensor_tensor(
                out=pack[:, :, 1], in0=pack[:, :, 1], scalar=1.0 / FN,
                in1=pp[:, :, 1], op0=MULT, op1=ADD)
            cin = dram.tile([128, nch, 2], F32, tag="ccin", name="cin")
            cout = dram.tile([128, nch, 2], F32, tag="ccout", name="cout")
            red = sbuf.tile([128, nch, 2], F32, tag="statred", name="red")
            nc.gpsimd.dma_start(out=cin, in_=pack)
            nc.gpsimd.collective_compute(
                "AllReduce", ADD, replica_groups=AR_GROUPS,
                ins=[cin.opt()], outs=[cout.opt()])
            nc.gpsimd.dma_start(out=red, in_=cout)
            emit_warm_burst(red[:, :, 0], wburst[0])
            return red

        def emit_s_t(red, nch, g_ap, b_ap):
            """s = g*rsqrt(var+eps), t = b - mean*s; red = [mean, E[x^2]]."""
            m = red[:, :, 0]
            v = sbuf.tile([128, nch], F32, tag="st_v", name="v")
            s = sbuf.tile([128, nch], F32, tag="st_s", name="s")
            t = sbuf.tile([128, nch], F32, tag="st_t", name="t")
            nc.vector.tensor_tensor(out=s, in0=m, in1=m, op=MULT)
            nc.vector.tensor_sub(v, red[:, :, 1], s)
            nc.scalar.activation(out=v, in_=v, func=Sqrt,
                                 bias=eps_t[:, 0:1], scale=1.0)
            nc.vector.reciprocal(out=s, in_=v)
            nc.vector.tensor_mul(s, s, g_ap)
            nc.vector.tensor_mul(v, m, s)
            nc.vector.tensor_sub(t, b_ap, v)
            return s, t

        def emit_warm_burst(dep_ap, wide_ap, n=9):
            """Fat junk matmuls gated on the AllReduce result: they stream
            N=512 at full PE-array duty during the post-barrier s/t/cast
            chain, tripping the HAM activity window so the first real matmuls
            issue at full clock instead of 1.2GHz."""
            redb = scratch.tile([128, KC], BF16, tag="redb", name="redb")
            nc.vector.tensor_copy(out=redb, in_=dep_ap)
            jp = psum.tile([KC, D], F32, tag="ps", name="jp")
            for _ in range(n):
                nc.tensor.matmul(jp, lhsT=redb, rhs=wide_ap,
                                 start=True, stop=True)

        def emit_w_cast(w_sb, s, name):
            """wf[:, kc, :] = w_sb[:, kc, :] * s[:, kc] -> bf16 [128, KC, D]."""
            wf = sbuf.tile([128, KC, D], BF16, tag="wf", name=name)
            for kc in range(KC):
                nc.vector.tensor_scalar_mul(
                    out=wf[:, kc, :], in0=w_sb[:, kc, :],
                    scalar1=s[:, kc:kc + 1])
            return wf

        def emit_bias_fold(wbf_chunks, t_ap, add_bias_ap, tag):
            """b' = t @ w + bias as [128, KC] via tiny bf16 PE matvecs."""
            nk = len(wbf_chunks)
            t_bf = sbuf.tile([128, nk], BF16, tag="tbf", name="t_bf")
            nc.vector.tensor_copy(out=t_bf, in_=t_ap)
            psb = paux.tile([128, KC], F32, tag="psb", name="psb")
            for dc in range(KC):
                for kc in range(nk):
                    nc.tensor.matmul(
                        psb[:, dc:dc + 1],
                        lhsT=wbf_chunks[kc][:, dc * 128:(dc + 1) * 128],
                        rhs=t_bf[:, kc:kc + 1],
                        start=(kc == 0), stop=(kc == nk - 1))
            bf = sbuf.tile([128, KC], F32, tag=tag, name=tag + "_bf")
            nc.vector.tensor_add(bf, psb, add_bias_ap)
            return bf

        def layer_matmuls(wf, j):
            """16 matmuls for node tile j: z[dc] = sum_kc wf[kc,dc].T @ R[kc,j]"""
            jsl = slice(j * NT, (j + 1) * NT)
            ps = []
            for dc in range(KC):
                p = psum.tile([128, NT], F32, tag="ps", name="p")
                for kc in range(KC):
                    nc.tensor.matmul(
                        p, lhsT=wf[:, kc, dc * 128:(dc + 1) * 128],
                        rhs=R[:, kc, jsl], start=(kc == 0), stop=(kc == KC - 1))
                ps.append(p)
            return ps

        def emit_mulreduce(in0, in1, accum, width, eng=None):
            """accum = sum(in0 * in1) along free axis (stt with dump)."""
            eng = eng or nc.vector
            dmp = scratch.tile([128, width], BF16, tag="qdump", name="dmp")
            eng.scalar_tensor_tensor(
                out=dmp[:, 0:width], in0=in0, scalar=1.0, in1=in1,
                op0=MULT, op1=MULT, accum_out=accum)

        def layer_drains(ps, bias_ap, Tacc, j, nact=3, acc_j=None):
            """Drain 4 psum chunks: relu+bias -> R[:, dc, jsl] (in place);
            ACT takes the first `nact` chunks (with sum accum), DVE the rest
            (stt with accum)."""
            jj = j if acc_j is None else acc_j
            jsl = slice(j * NT, (j + 1) * NT)
            for dc in range(nact):
                acc = Tacc[:, dc, jj:jj + 1] if Tacc is not None else None
                nc.scalar.activation(
                    out=R[:, dc, jsl], in_=ps[dc], func=Relu,
                    bias=bias_ap[:, dc:dc + 1], scale=1.0, accum_out=acc)
            for dc in range(nact, KC):
                acc = Tacc[:, dc, jj:jj + 1] if Tacc is not None else None
                nc.vector.scalar_tensor_tensor(
                    out=R[:, dc, jsl], in0=ps[dc],
                    scalar=bias_ap[:, dc:dc + 1], in1=zeros_t,
                    op0=ADD, op1=MAX, accum_out=acc)

        def emit_q_spans(j, Qacc, base=0, ntiles=NTILES):
            """After tile j's drains: sumsq over the completed 2-tile span."""
            jj = j - base
            if jj % 2 == 1 or jj == ntiles - 1:
                sp = jj // 2
                a = (j - 1) * NT if jj % 2 == 1 else j * NT
                b = (j + 1) * NT
                for dc in range(KC):
                    emit_mulreduce(R[:, dc, a:b], R[:, dc, a:b],
                                   Qacc[:, dc, sp:sp + 1], b - a)

        NQSP = (NTILES + 1) // 2  # 13 sumsq spans per layer

        # ---------- resident hidden buffer (bf16, holds r0 -> r1 -> r2 -> r3)
        R = sbuf.tile([128, KC, NS], BF16, tag="R", name="R")

        # ---------- constants + activation-table preload ---------------------
        eps_t = sbuf.tile([128, 1], F32, tag="eps", name="eps_t")
        nc.vector.memset(eps_t, EPS)
        zeros_t = sbuf.tile([128, NT], F32, tag="zeros", name="zeros_t")
        nc.vector.memset(zeros_t, 0.0)
        ident = sbuf.tile([128, 128], F32, tag="ident", name="ident")
        identG = sbuf.tile([G, G], F32, tag="identG", name="identG")
        identGb = sbuf.tile([G, G], BF16, tag="identGb", name="identGb")
        ones_t = sbuf.tile([128, 128], BF16, tag="ones", name="ones_t")
        # only Relu+Sqrt tables up front (Sigmoid is prewarmed mid-L3);
        # extra preloads would stall ACT right when the first drains arrive.
        for fn in (Relu, Sqrt):
            dmy = scratch.tile([128, 1], F32, tag="dummy", name="dmy")
            nc.scalar.activation(out=dmy, in_=eps_t, func=fn)


        qconst = nc.gpsimd if ALT_DMA_Q else nc.sync

        def load_const(key, shape, tag):
            t = sbuf.tile(shape, F32, tag=tag, name=tag)
            qconst.dma_start(out=t, in_=ins[key])
            return t

        # ================= P0: x streaming (bn1 folded on host) ==============
        # Sync queue priority: w0 -> x span0 -> w1 -> x spans 1-4 (the first
        # matmul gates on w0 + span0 only); consts on GpSimd (b0c first, it
        # gates the first drains).
        with tc.tile_pool(name="w01pool", bufs=1) as w01pool, \
             tc.tile_pool(name="psumB", bufs=2, space="PSUM") as psumB:
            x_bf = w01pool.tile([128, NS], BF16, tag="xbf", name="x_bf")
            nc.sync.dma_start(out=x_bf[:, 0:NT], in_=ins["xT"][:, 0:NT])
            w0_sb = w01pool.tile([128, D], F32, tag="w0", name="w0_sb")
            nc.sync.dma_start(out=w0_sb, in_=ins["w0"])
            nc.sync.dma_start(out=x_bf[:, NT:XSP], in_=ins["xT"][:, NT:XSP])
            nc.sync.dma_start(out=x_bf[:, XSP:2 * XSP],
                              in_=ins["xT"][:, XSP:2 * XSP])
            w1_sb = wstage.tile([128, KC, D], F32, tag="wst", name="w1_sb")
            for kc in range(KC):
                nc.sync.dma_start(out=w1_sb[:, kc, :],
                                  in_=ins["w1"][kc * 128:(kc + 1) * 128, :])
            for sp in range(2, NXSP):
                a = sp * XSP
                nc.sync.dma_start(out=x_bf[:, a:a + XSP],
                                  in_=ins["xT"][:, a:a + XSP])

            b0c = load_const("b0c", [128, KC], "b0c")
            bb1c = load_const("bb1c", [128, KC], "bb1c")
            bn3g = load_const("bn3g", [128, KC], "bn3g")
            bn3b = load_const("bn3b", [128, KC], "bn3b")
            bb2c = load_const("bb2c", [128, KC], "bb2c")
            bb3c = load_const("bb3c", [128, KC], "bb3c")

            w0f = w01pool.tile([128, D], BF16, tag="wbf0", name="w0f")
            for dc in range(KC):
                nc.vector.tensor_copy(out=w0f[:, dc * 128:(dc + 1) * 128],
                                      in_=w0_sb[:, dc * 128:(dc + 1) * 128])

            # absorb first-collective entry sync (cross-core launch skew)
            # with a throwaway AllReduce fired at t~0 on the GpSimd queue.
            dmy_in = dram.tile([8], F32, tag="dmyi", name="dmy_in")
            dmy_out = dram.tile([8], F32, tag="dmyo", name="dmy_out")
            dmy_sb = scratch.tile([1, 8], F32, tag="dmysb", name="dmy_sb")
            nc.gpsimd.memset(dmy_sb, 0.0)
            nc.gpsimd.dma_start(out=dmy_in.rearrange("(a b) -> a b", a=1),
                                in_=dmy_sb)
            nc.gpsimd.collective_compute(
                "AllReduce", ADD, replica_groups=AR_GROUPS,
                ins=[dmy_in.opt()], outs=[dmy_out.opt()])

            # plain bf16 cast of w1 (layer-1 input r0 has no preceding BN)
            w1f = w01pool.tile([128, KC, D], BF16, tag="w1f", name="w1f")
            for kc in range(KC):
                nc.vector.tensor_copy(out=w1f[:, kc, :], in_=w1_sb[:, kc, :])

            # identities / ones: emitted after the startup-critical DMAs and
            # casts so they stall neither the GpSimd const queue nor the DVE
            # queue ahead of the first drains (used from the barriers on).
            from concourse.masks import make_identity
            make_identity(nc, ident)
            make_identity(nc, identG)
            nc.vector.tensor_copy(out=identGb, in_=identG)
            nc.vector.memset(ones_t, 1.0)

            # stage w2 (slot WAR clears after the w1f cast, still in P0)
            w2_sb = wstage.tile([128, KC, D], F32, tag="wst", name="w2_sb")
            for kc in range(KC):
                nc.sync.dma_start(out=w2_sb[:, kc, :],
                                  in_=ins["w2"][kc * 128:(kc + 1) * 128, :])
            # small fc weights stream during P1 (fresh space, no WAR)
            fc2w_sb = wstage.tile([128, KC, 256], F32, tag="wst2",
                                  name="fc2w_sb")
            for kc in range(KC):
                nc.sync.dma_start(out=fc2w_sb[:, kc, :],
                                  in_=ins["fc2w"][kc * 128:(kc + 1) * 128, :])
            fc3w_sb = wstage.tile([128, 2, C], F32, tag="wst3", name="fc3w_sb")
            for kc in range(2):
                nc.sync.dma_start(out=fc3w_sb[:, kc, :],
                                  in_=ins["fc3w"][kc * 128:(kc + 1) * 128, :])

            # ---- P1: fused L0+L1, software-pipelined by one tile ------------
            T1 = sbuf.tile([128, KC, NTILES], F32, tag="T1", name="T1")
            Q1 = sbuf.tile([128, KC, NQSP], F32, tag="Q1", name="Q1")

            def emit_l0(j):
                jsl = slice(j * NT, (j + 1) * NT)
                ps0 = []
                for dc in range(KC):
                    p = psumB.tile([128, NT], F32, tag="ps0", name="p0")
                    nc.tensor.matmul(
                        p, lhsT=w0f[:, dc * 128:(dc + 1) * 128],
                        rhs=x_bf[:, jsl], start=True, stop=True)
                    ps0.append(p)
                for dc in range(2):
                    nc.scalar.activation(
                        out=R[:, dc, jsl], in_=ps0[dc], func=Relu,
                        bias=b0c[:, dc:dc + 1], scale=1.0)
                for dc in range(2, KC):
                    nc.vector.tensor_scalar(
                        out=R[:, dc, jsl], in0=ps0[dc],
                        scalar1=b0c[:, dc:dc + 1], scalar2=0.0,
                        op0=ADD, op1=MAX)

            def emit_l1(jj):
                ps = layer_matmuls(w1f, jj)
                layer_drains(ps, bb1c, T1, jj,
                             nact=4 if jj == NTILES - 1 else 3)
                emit_q_spans(jj, Q1)

            wburst.append(w1f[:, 0, :])
            pp1 = None
            if FUSE_P1:
                for j in range(NTILES + 1):
                    if j < NTILES:
                        emit_l0(j)
                    if j >= 1:
                        emit_l1(j - 1)
                    if j == NTILES - 2:
                        pp1 = stats_prepack(T1, Q1, KC, NTILES - 2, NQSP - 2)
            else:
                for j in range(NTILES):
                    emit_l0(j)
                for j in range(NTILES):
                    emit_l1(j)
                pp1 = stats_prepack(T1, Q1, KC, NTILES - 1, NQSP - 1)

        # ================= barrier #2 + L2 ===================================
        red2 = stats_allreduce(pp1, T1, Q1, KC, NTILES - 2, NQSP - 2)
        s2, t2 = emit_s_t(red2, KC, bn3g, bn3b)
        w2f = emit_w_cast(w2_sb, s2, "w2f")
        wbf2 = sbuf.tile([128, KC, D], BF16, tag="wbf", name="wbf2")
        for kc in range(KC):
            nc.vector.tensor_copy(out=wbf2[:, kc, :], in_=w2_sb[:, kc, :])
        b2f = emit_bias_fold([wbf2[:, kc, :] for kc in range(KC)],
                             t2, bb2c, "b2f")
        # stage w3 (slot WAR clears right here, at the w2 casts)
        w3_sb = wstage.tile([128, KC, D], F32, tag="wst", name="w3_sb")
        for kc in range(KC):
            nc.sync.dma_start(out=w3_sb[:, kc, :],
                              in_=ins["w3"][kc * 128:(kc + 1) * 128, :])
        T2 = sbuf.tile([128, KC, NTILES], F32, tag="T2", name="T2")
        Q2 = sbuf.tile([128, KC, NQSP], F32, tag="Q2", name="Q2")
        pp2 = None
        for j in range(NTILES):
            ps = layer_matmuls(w2f, j)
            layer_drains(ps, b2f, T2, j, nact=4 if j == NTILES - 1 else 3)
            emit_q_spans(j, Q2)
            if j == NTILES - 3:
                pp2 = stats_prepack(T2, Q2, KC, NTILES - 2, NQSP - 2)

        # ================= barrier #3 + L3 + pooling + FC head ===============
        NPURE = NTILES - KMIX
        NQ3 = (NPURE + 1) // 2
        with tc.tile_pool(name="fcpool", bufs=1) as fcpool, \
             tc.tile_pool(name="pmix", bufs=1, space="PSUM") as pmix:
            # tail consts on the Sync DMA queue -- the GpSimd queue must stay
            # clear so the stats-AllReduce triggers fire promptly.
            arm_sb = fcpool.tile([NPURE, G], F32, tag="arm", name="arm_sb")
            nc.sync.dma_start(out=arm_sb, in_=ins["arm"])
            oh_sb = fcpool.tile([125, KMIX, KC, G], BF16, tag="oh", name="oh_sb")
            nc.sync.dma_start(out=oh_sb, in_=ins["oh"])
            ngb = fcpool.tile([128, G], F32, tag="ngb", name="ngb")
            nc.sync.dma_start(out=ngb, in_=ins["cnts"].to_broadcast([128, G]))
            a3b = fcpool.tile([128, 1], F32, tag="a3b", name="a3b")
            nc.sync.dma_start(out=a3b, in_=ins["a3v"].to_broadcast([128, 1]))
            fc1bc_sb = fcpool.tile([128, KC], F32, tag="fc1bc", name="fc1bc_sb")
            nc.sync.dma_start(out=fc1bc_sb, in_=ins["fc1bc"])
            fc2bc_sb = fcpool.tile([128, 2], F32, tag="fc2bc", name="fc2bc_sb")
            nc.sync.dma_start(out=fc2bc_sb, in_=ins["fc2bc"])
            fc3bc_sb = fcpool.tile([C, 1], F32, tag="fc3bc", name="fc3bc_sb")
            nc.sync.dma_start(out=fc3bc_sb, in_=ins["fc3bc"])
            bb3r_sb = fcpool.tile([1, D], F32, tag="bb3r", name="bb3r_sb")
            nc.sync.dma_start(out=bb3r_sb, in_=ins["bb3r"])

            red3 = stats_allreduce(pp2, T2, Q2, KC, NTILES - 2, NQSP - 2)
            s3, t3 = emit_s_t(red3, KC, bn3g, bn3b)
            w3f = emit_w_cast(w3_sb, s3, "w3f")
            wbf3 = sbuf.tile([128, KC, D], BF16, tag="wbf", name="wbf3")
            for kc in range(KC):
                nc.vector.tensor_copy(out=wbf3[:, kc, :], in_=w3_sb[:, kc, :])
            b3f = emit_bias_fold([wbf3[:, kc, :] for kc in range(KC)],
                                 t3, bb3c, "b3f")
            # row layout of the same folded bias, for the node-major mixed path
            t3b = fcpool.tile([128, KC], BF16, tag="t3b", name="t3b")
            nc.vector.tensor_copy(out=t3b, in_=t3)
            pbr = pmix.tile([1, D], F32, tag="pbr", name="pbr")
            for kc in range(KC):
                nc.tensor.matmul(pbr, lhsT=t3b[:, kc:kc + 1],
                                 rhs=wbf3[:, kc, :],
                                 start=(kc == 0), stop=(kc == KC - 1))
            b3row = fcpool.tile([1, D], BF16, tag="b3row", name="b3row")
            nc.vector.tensor_tensor(out=b3row, in0=pbr, in1=bb3r_sb, op=ADD)
            # stage fc1w (slot WAR clears at the w3 casts); bf16 cast follows
            fc1w_sb = wstage.tile([128, KC, D], F32, tag="wst", name="fc1w_sb")
            for kc in range(KC):
                nc.sync.dma_start(out=fc1w_sb[:, kc, :],
                                  in_=ins["fc1w"][kc * 128:(kc + 1) * 128, :])

            # ---- L3 mixed tiles: node-major z3^T path (emitted after two
            # dense pure tiles so the PE is back at full clock first) --------
            # z3t[n,f] = sum_kc r2[kc,n]^T w3f[kc,f] (+ 1 x b3row), relu on ACT,
            # then pooled directly per graph (oh) and sumsq via a ones matvec;
            # no transposes, and the whole tail dependence shrinks to the AR.
            mix = {}

            def emit_mixed_phase():
                poolM = pmix.tile([G, D], F32, tag="poolM", name="poolM")
                sqrow = pmix.tile([1, D], F32, tag="pbr", name="sqrow")
                nmm = KMIX * KC
                pend = []

                def emit_mix_pool(p):
                    r3t_, sq_, k_, pc_, i_ = p
                    nc.tensor.matmul(poolM, lhsT=oh_sb[:, k_, pc_, :],
                                     rhs=r3t_,
                                     start=(i_ == 0), stop=(i_ == nmm - 1),
                                     skip_group_check=True)
                    nc.tensor.matmul(sqrow, lhsT=ones_t[0:125, 0:1], rhs=sq_,
                                     start=(i_ == 0), stop=(i_ == nmm - 1),
                                     skip_group_check=True)

                for k in range(KMIX):
                    for pc in range(KC):
                        a = k * NT + pc * 125
                        zt = psum.tile([125, D], F32, tag="ps", name="zt")
                        for kc in range(KC):
                            nc.tensor.matmul(zt, lhsT=R[:, kc, a:a + 125],
                                             rhs=w3f[:, kc, :],
                                             start=(kc == 0), stop=False)
                        nc.tensor.matmul(zt, lhsT=ones_t[0:1, 0:125],
                                         rhs=b3row, start=False, stop=True)
                        r3t = scratch.tile([125, D], BF16, tag="r3t",
                                           name="r3t")
                        nc.scalar.activation(out=r3t, in_=zt, func=Relu,
                                             bias=zeros_t[0:125, 0:1],
                                             scale=1.0)
                        sq = scratch.tile([125, D], BF16, tag="sqm", name="sq")
                        nc.vector.tensor_tensor(out=sq, in0=r3t, in1=r3t,
                                                op=MULT)
                        pend.append((r3t, sq, k, pc, k * KC + pc))
                        if len(pend) > 2:
                            emit_mix_pool(pend.pop(0))
                for p in pend:
                    emit_mix_pool(p)

                # mixed-node sumsq -> feature-major (runs during pure L3)
                sq_sb = fcpool.tile([1, D], F32, tag="sqsb", name="sq_sb")
                nc.vector.tensor_copy(out=sq_sb, in_=sqrow)
                poolM_bf = fcpool.tile([G, D], F32, tag="poolMsb",
                                       name="poolM_bf")
                nc.vector.tensor_copy(out=poolM_bf, in_=poolM)
                sqT = paux.tile([128, KC], F32, tag="psb", name="sqT")
                for kc in range(KC):
                    nc.tensor.matmul(sqT[:, kc:kc + 1],
                                     lhsT=sq_sb[0:1, kc * 128:(kc + 1) * 128],
                                     rhs=ident[0:1, 0:1], start=True,
                                     stop=True)
                # bf16 fc-weight copies, emitted here so they clear the DVE
                # queue long before the stats pack of the last pure tile
                fc1wb = fcpool.tile([128, KC, D], BF16, tag="fc1wb",
                                    name="fc1wb")
                for kc in range(KC):
                    nc.vector.tensor_copy(out=fc1wb[:, kc, :],
                                          in_=fc1w_sb[:, kc, :])
                fc2wb = fcpool.tile([128, KC, 256], BF16, tag="fc2wb",
                                    name="fc2wb")
                for kc in range(KC):
                    nc.vector.tensor_copy(out=fc2wb[:, kc, :],
                                          in_=fc2w_sb[:, kc, :])
                fc3wb = fcpool.tile([128, 2, C], BF16, tag="fc3wb",
                                    name="fc3wb")
                for kc in range(2):
                    nc.vector.tensor_copy(out=fc3wb[:, kc, :],
                                          in_=fc3w_sb[:, kc, :])
                mix.update(sqT=sqT, poolM_bf=poolM_bf, fc1wb=fc1wb,
                           fc2wb=fc2wb, fc3wb=fc3wb)

            # ---- L3 pure tiles: r3 -> R in place; per-tile sums + sumsq -----
            T3 = sbuf.tile([128, KC, NPURE], F32, tag="TC", name="T3")
            Q3 = sbuf.tile([128, KC, NQ3], F32, tag="Q3", name="Q3")
            pp3 = None
            for jj in range(NPURE):
                if jj == 2:
                    emit_mixed_phase()
                j = KMIX + jj
                ps = layer_matmuls(w3f, j)
                layer_drains(ps, b3f, T3, j, acc_j=jj,
                             nact=4 if jj == NPURE - 1 else 3)
                emit_q_spans(j, Q3, base=KMIX, ntiles=NPURE)
                if jj == 2 * NQ3 - 5:
                    # partial sumsq (pure spans + mixed), pre-scaled by 1/N
                    q1 = sbuf.tile([128, KC], F32, tag="pp3a", name="q1")
                    nc.vector.tensor_reduce(out=q1, in_=Q3[:, :, 0:NQ3 - 2],
                                            axis=AXX, op=ADD)
                    nc.vector.tensor_add(q1, q1, mix['sqT'])
                    pp3 = sbuf.tile([128, KC], F32, tag="pp3", name="pp3")
                    nc.vector.tensor_scalar_mul(out=pp3, in0=q1,
                                                scalar1=1.0 / FN)

            # pooled partials: arm.T @ T3.T (pure, fp32) + poolM folded into
            # the same psum via an identity matmul; psum -> SBUF copies ride
            # the ACT engine (values are sums of relus, so Relu == Copy) and
            # each dc's slice DMAs to the collective buffer as soon as ready.
            nst = 128 * KC
            flat = dram.tile([nst + G * D], F32, tag="cc4in", name="flat")
            flat_out = dram.tile([nst + G * D], F32, tag="cc4out",
                                 name="flat_out")
            flat2 = flat[nst:].rearrange("(g f) -> g f", g=G)
            pack = sbuf.tile([128, KC], F32, tag="statpack", name="pack4")
            nc.vector.tensor_add(pack, Q3[:, :, NQ3 - 2], Q3[:, :, NQ3 - 1])
            nc.vector.scalar_tensor_tensor(
                out=pack, in0=pack, scalar=1.0 / FN,
                in1=pp3, op0=MULT, op1=ADD)
            nc.gpsimd.dma_start(
                out=flat[0:nst].rearrange("(p c) -> p c", p=128), in_=pack)
            poolGc = fcpool.tile([G, D], F32, tag="poolG", name="poolGc")
            for dc in range(KC):
                dsl = slice(dc * 128, (dc + 1) * 128)
                pT = psum.tile([NPURE, 128], F32, tag="ps", name="pT")
                nc.tensor.transpose(pT, T3[:, dc, :], ident)
                tct = scratch.tile([NPURE, 128], F32, tag="tct", name="tct")
                nc.scalar.activation(out=tct, in_=pT, func=Relu)
                pG = psum.tile([G, 128], F32, tag="ps", name="pG")
                nc.tensor.matmul(pG, lhsT=arm_sb, rhs=tct,
                                 start=True, stop=False)
                nc.tensor.matmul(pG, lhsT=identG, rhs=mix['poolM_bf'][:, dsl],
                                 start=False, stop=True)
                nc.scalar.activation(out=poolGc[:, dsl], in_=pG, func=Relu)
                qd = nc.scalar if dc < 2 else nc.sync
                qd.dma_start(out=flat2[:, dsl], in_=poolGc[:, dsl])
            nc.gpsimd.collective_compute(
                "AllReduce", ADD, replica_groups=AR_GROUPS,
                ins=[flat.opt()], outs=[flat_out.opt()])
            red4 = sbuf.tile([128, KC, 2], F32, tag="statred", name="red4")
            poolGr = fcpool.tile([G, D], F32, tag="poolGr", name="poolGr")
            nc.gpsimd.dma_start(
                out=red4[:, :, 1],
                in_=flat_out[0:nst].rearrange("(p c) -> p c", p=128))
            emit_warm_burst(red4[:, :, 1], mix['fc1wb'][:, 0, :], n=8)
            flat2o = flat_out[nst:].rearrange("(g f) -> g f", g=G)

            # mean comes free from the pooled sums: m = sum_g pooled / N
            pFs = []
            for dc in range(KC):
                dsl = slice(dc * 128, (dc + 1) * 128)
                qd = nc.scalar if dc < 2 else nc.sync
                qd.dma_start(out=poolGr[:, dsl], in_=flat2o[:, dsl])
                pF = psum.tile([128, G], F32, tag="ps", name="pF")
                nc.tensor.transpose(pF, poolGr[:, dsl], identG)
                nc.vector.tensor_reduce(out=red4[:, dc:dc + 1, 0], in_=pF,
                                        axis=AXX, op=ADD)
                pFs.append(pF)
            nc.vector.tensor_scalar_mul(out=red4[:, :, 0], in0=red4[:, :, 0],
                                        scalar1=1.0 / FN)
            s4, t4 = emit_s_t(red4, KC, bn3g, bn3b)
            dmy2 = scratch.tile([128, 1], F32, tag="dummy", name="dmy2")
            nc.scalar.activation(out=dmy2, in_=eps_t, func=Sig)

            # pooled_bn[f, g] = s4[f]*pooled[f, g] + t4[f]*n[g] (feature-major,
            # bf16 for the fc matmuls)
            pooled = fcpool.tile([128, KC, G], BF16, tag="pooled", name="pooled")
            for dc in range(KC):
                ngt = scratch.tile([128, G], F32, tag="ngt", name="ngt")
                nc.vector.tensor_scalar(
                    out=ngt, in0=ngb, scalar1=t4[:, dc:dc + 1],
                    scalar2=None, op0=MULT)
                nc.vector.scalar_tensor_tensor(
                    out=pooled[:, dc, :], in0=pFs[dc],
                    scalar=s4[:, dc:dc + 1], in1=ngt, op0=MULT, op1=ADD)

            # ---------------- FC head (bf16 matmuls) -------------------------
            h1 = fcpool.tile([128, KC, G], BF16, tag="h1", name="h1")
            for dc in range(KC):
                p = psum.tile([128, G], F32, tag="ps", name="pfc")
                for kc in range(KC):
                    nc.tensor.matmul(
                        p, lhsT=mix['fc1wb'][:, kc, dc * 128:(dc + 1) * 128],
                        rhs=pooled[:, kc, :], start=(kc == 0), stop=(kc == KC - 1))
                z = scratch.tile([128, G], F32, tag="fcz", name="z")
                nc.vector.tensor_scalar(out=z, in0=p,
                                        scalar1=fc1bc_sb[:, dc:dc + 1],
                                        scalar2=None, op0=ADD)
                # prelu(z) = max(a*z, z) for 0 <= a <= 1
                nc.vector.scalar_tensor_tensor(
                    out=h1[:, dc, :], in0=z, scalar=a3b[:, 0:1], in1=z,
                    op0=MULT, op1=MAX)

            # fc2 + sigmoid -> h2 [128, 2, G]
            h2 = fcpool.tile([128, 2, G], BF16, tag="h2", name="h2")
            for ec in range(2):
                p = psum.tile([128, G], F32, tag="ps", name="pfc2")
                for kc in range(KC):
                    nc.tensor.matmul(
                        p, lhsT=mix['fc2wb'][:, kc, ec * 128:(ec + 1) * 128],
                        rhs=h1[:, kc, :], start=(kc == 0), stop=(kc == KC - 1))
                nc.scalar.activation(out=h2[:, ec, :], in_=p, func=Sig,
                                     bias=fc2bc_sb[:, ec:ec + 1], scale=1.0)

            # fc3 -> out [10, 64]
            p = psum.tile([C, G], F32, tag="ps", name="pfc3")
            for kc in range(2):
                nc.tensor.matmul(p, lhsT=mix['fc3wb'][:, kc, :], rhs=h2[:, kc, :],
                                 start=(kc == 0), stop=(kc == 1))
            ob = fcpool.tile([C, G], F32, tag="ob", name="ob")
            nc.vector.tensor_scalar(out=ob, in0=p, scalar1=fc3bc_sb,
                                    scalar2=None, op0=ADD)
            nc.sync.dma_start(out=out_ap, in_=ob)


_cached = {}


def kernel(**inputs) -> np.ndarray:
    in_maps, kmix = _build_host_inputs(inputs)
    if _cached.get("kmix") != kmix:
        _cached["nc"] = build_program(kmix)
        _cached["kmix"] = kmix
    nc = _cached["nc"]
    res = bass_utils.run_bass_kernel_spmd(
        nc, in_maps, core_ids=list(range(NCORES)))
    out = res.results[0]["out"]  # [10, 64]
    return np.ascontiguousarray(out.T.astype(np.float32))


if __name__ == "__main__":
    import reference
    inp = {k: np.asarray(v) for k, v in reference.setup_inputs().items()}
    got = kernel(**inp)
    exp = np.asarray(reference.reference(**{
        k: np.asarray(v) for k, v in reference.setup_inputs().items()}))
    err = np.linalg.norm(got - exp) / np.linalg.norm(exp)
    print("Relative error:", err)

